# revision 12
# baseline (speedup 1.0000x reference)
"""Causal self-attention (B=4, T=2048, C=768, H=12, D=64) on 8 TRN2 NeuronCores.

Sharding: core = 2*b + hg. Data parallel over batch (4), tensor parallel over
heads (2 groups of 6). Each core computes qkv for its 6 heads, causal
attention, and a partial output projection (its heads' columns of w_proj);
the host sums the two partials per batch and adds b_proj.

Layout notes (per core):
  - xT   [768, 2048]  x[b] transposed on host (contraction dim on partitions)
  - kq   [128, 6, 2048] SBUF: f-tiles 0-2 = K^T feats, 3-5 = Q^T feats
  - v    [128, 16, 390] SBUF: token-major V, 65 cols/head (col 64 = ones so
         the attn@V matmul also produces the softmax denominator l)
  - scores computed transposed S^T[k, q] so no transposes are needed anywhere;
    softmax uses no max-subtraction (logits are O(10) for this problem) so
    P = exp(0.25 * QK^T_raw), Y^T_aug = V_aug^T @ P^T accumulated over k-tiles.
  - matmuls run as float32r (full fp32 data, 1 cycle/row when N>=256).
"""
import sys

for _p in ("/opt/trn_rl_repo",):
    if _p not in sys.path:
        sys.path.append(_p)

import numpy as np

B, T, C = 4, 2048, 768
H, D = 12, 64
HL = H // 2          # 6 local heads
FL = HL * D          # 384 local features
NCT = C // 128       # 6 contraction tiles
NTT = T // 128       # 16 token tiles
QCH = 512            # q chunk (free dim of attention matmuls)
NQC = T // QCH       # 4 q chunks
VW = D + 1           # 65: V columns per head incl. ones column
EXP_SCALE = 2.0 / np.sqrt(D)  # reference uses logits = 2 * scores / sqrt(D)

_cache = {}


def _build(nc_mod=None):
    import concourse.bass as bass
    import concourse.tile as tile
    from concourse import bacc, mybir

    f32 = mybir.dt.float32
    f32r = mybir.dt.float32r
    Exp = mybir.ActivationFunctionType.Exp

    nc = bacc.Bacc("TRN2", target_bir_lowering=False, debug=False, num_devices=8)

    xT = nc.dram_tensor("xT", [C, T], f32r, kind="ExternalInput").ap()
    wkqT = nc.dram_tensor("wkqT", [C, 2 * FL], f32r, kind="ExternalInput").ap()
    wvT = nc.dram_tensor("wvT", [C, FL], f32r, kind="ExternalInput").ap()
    bkq = nc.dram_tensor("bkq", [2 * FL], f32, kind="ExternalInput").ap()
    bv = nc.dram_tensor("bv", [FL], f32, kind="ExternalInput").ap()
    wpT = nc.dram_tensor("wpT", [FL, C], f32r, kind="ExternalInput").ap()
    out = nc.dram_tensor("out", [T, C], f32, kind="ExternalOutput").ap()

    def r(ap):
        return ap  # tiles are already float32r

    with tile.TileContext(nc) as tc:
        from contextlib import ExitStack

        with ExitStack() as ctx:
            persist = ctx.enter_context(tc.tile_pool(name="persist", bufs=1))
            xpool = ctx.enter_context(tc.tile_pool(name="xchunk", bufs=2))
            ppool = ctx.enter_context(tc.tile_pool(name="ptile", bufs=3))
            lpool = ctx.enter_context(tc.tile_pool(name="linv", bufs=2))
            lrpool = ctx.enter_context(tc.tile_pool(name="linvrep", bufs=2))
            opool = ctx.enter_context(tc.tile_pool(name="outstg", bufs=2))
            ps_mm = ctx.enter_context(tc.tile_pool(name="psmm", bufs=2, space="PSUM"))
            ps_s = ctx.enter_context(tc.tile_pool(name="pss", bufs=3, space="PSUM"))
            ps_y = ctx.enter_context(tc.tile_pool(name="psy", bufs=2, space="PSUM"))
            ps_b = ctx.enter_context(tc.tile_pool(name="psb", bufs=1, space="PSUM"))

            # ---- persistent SBUF tensors ----
            kq_sb = persist.tile([128, 6, T], f32r)        # K^T (0-2) / Q^T (3-5)
            v_sb = persist.tile([128, NTT, HL * VW], f32r)  # token-major V + ones
            yn_sb = persist.tile([128, 3, T], f32r)         # normalized Y^T
            wkq_sb = persist.tile([128, NCT, 2 * FL], f32r)
            wv_sb = persist.tile([128, NCT, FL], f32r)
            wp_sb = persist.tile([128, 3, C], f32r)
            bkq_sb = persist.tile([128, NCT], f32)
            bv_rep = persist.tile([128, FL], f32)
            masks = persist.tile([128, 4, QCH], f32r)
            ones_sb = persist.tile([1, 64], f32r)
            nc.vector.memset(ones_sb.bitcast(f32), 1.0)

            # ---- load weights / biases ----
            nc.sync.dma_start(
                out=wkq_sb, in_=wkqT.rearrange("(a p) f -> p a f", p=128)
            )
            nc.sync.dma_start(out=wv_sb, in_=wvT.rearrange("(a p) f -> p a f", p=128))
            nc.sync.dma_start(out=wp_sb, in_=wpT.rearrange("(a p) f -> p a f", p=128))
            nc.sync.dma_start(out=bkq_sb, in_=bkq.rearrange("(a p) -> p a", p=128))
            nc.gpsimd.dma_start(
                out=bv_rep,
                in_=bass.AP(tensor=bv.tensor, offset=0, ap=[[0, 128], [1, FL]]),
            )

            # ---- causal masks for the 4 diagonal offsets ----
            # tile (kt, qc) is diagonal when mi = kt - qc*4 in {0,1,2,3}; its
            # global offset is qc*512 - kt*128 = -mi*128, and an element
            # (p=k rel, f=q rel) is valid (keep) iff -mi*128 + f - p >= 0.
            for mi in range(4):
                m = masks[:, mi, :].bitcast(f32)
                nc.gpsimd.memset(m, 1.0)
                nc.gpsimd.affine_select(
                    out=m,
                    in_=m,
                    compare_op=mybir.AluOpType.is_ge,
                    fill=0.0,
                    base=-mi * 128,
                    channel_multiplier=-1,
                    pattern=[[1, QCH]],
                )

            # ones columns of v_sb
            v4 = v_sb.rearrange("p t (h w) -> p t h w", h=HL)
            nc.vector.memset(v4[:, :, :, D : D + 1].bitcast(f32), 1.0)

            # ---- phase A: qkv ----
            for tn in range(NQC):  # token chunks of 512
                xt = xpool.tile([128, NCT, QCH], f32r)
                nc.sync.dma_start(
                    out=xt,
                    in_=xT.rearrange("(a p) t -> p a t", p=128)[
                        :, :, tn * QCH : (tn + 1) * QCH
                    ],
                )
                # K^T / Q^T: out[f_tile, tok]
                for fj in range(6):
                    ps = ps_mm.tile([128, QCH], f32, tag="psmm")
                    for ci in range(NCT):
                        nc.tensor.matmul(
                            ps,
                            lhsT=r(wkq_sb[:, ci, fj * 128 : (fj + 1) * 128]),
                            rhs=r(xt[:, ci, :]),
                            start=(ci == 0),
                            stop=(ci == NCT - 1),
                        )
                    nc.vector.tensor_scalar_add(
                        kq_sb[:, fj, tn * QCH : (tn + 1) * QCH],
                        ps,
                        bkq_sb[:, fj : fj + 1],
                    )
                # V: out[token, f] (+ bias along free dim)
                for k4 in range(4):
                    kt = tn * 4 + k4
                    ps = ps_mm.tile([128, FL], f32, tag="psmm")
                    for ci in range(NCT):
                        nc.tensor.matmul(
                            ps,
                            lhsT=r(xt[:, ci, k4 * 128 : (k4 + 1) * 128]),
                            rhs=r(wv_sb[:, ci, :]),
                            start=(ci == 0),
                            stop=(ci == NCT - 1),
                        )
                    nc.vector.tensor_add(
                        v4[:, kt, :, 0:D],
                        ps.rearrange("p (h d) -> p h d", h=HL),
                        bv_rep.rearrange("p (h d) -> p h d", h=HL),
                    )

            # ---- phase B: attention, phase C: projection per q-chunk ----
            for qc in range(NQC):
                for h in range(HL):
                    hp, hi = h % 2, h // 2
                    kmax = (qc + 1) * 4
                    psy = ps_y.tile([128, QCH], f32, tag="psy")
                    for kt in range(kmax):
                        pss = ps_s.tile([128, QCH], f32, tag="pss")
                        nc.tensor.matmul(
                            pss,
                            lhsT=r(
                                kq_sb[
                                    hp * 64 : hp * 64 + 64,
                                    hi,
                                    kt * 128 : (kt + 1) * 128,
                                ]
                            ),
                            rhs=r(
                                kq_sb[
                                    hp * 64 : hp * 64 + 64,
                                    3 + hi,
                                    qc * QCH : (qc + 1) * QCH,
                                ]
                            ),
                            start=True,
                            stop=True,
                        )
                        pt = ppool.tile([128, QCH], f32r, tag="ptile")
                        nc.scalar.activation(pt, pss, Exp, scale=float(EXP_SCALE))
                        mi = kt - qc * 4
                        if mi >= 0:  # diagonal tile -> apply causal mask
                            nc.vector.tensor_mul(pt, pt, masks[:, mi, :])
                        nc.tensor.matmul(
                            psy[0:VW, :],
                            lhsT=r(v_sb[:, kt, h * VW : (h + 1) * VW]),
                            rhs=r(pt),
                            start=(kt == 0),
                            stop=(kt == kmax - 1),
                        )
                    linv = lpool.tile([1, QCH], f32r, tag="linv")
                    with nc.allow_low_precision(reason="f32r is fp32-width"):
                        nc.vector.reciprocal(linv, psy[D : D + 1, :])
                    psb = ps_b.tile([64, QCH], f32, tag="psb")
                    nc.tensor.matmul(
                        psb, lhsT=r(ones_sb), rhs=r(linv), start=True, stop=True
                    )
                    lrep = lrpool.tile([64, QCH], f32, tag="linvrep")
                    nc.vector.tensor_copy(lrep, psb)
                    nc.vector.tensor_mul(
                        yn_sb[hp * 64 : hp * 64 + 64, hi, qc * QCH : (qc + 1) * QCH],
                        psy[0:D, :],
                        lrep,
                    )
                # projection for this q-chunk (all heads now normalized)
                for q4 in range(4):
                    qt = qc * 4 + q4
                    ostg = opool.tile([128, C], f32, tag="outstg")
                    for cj in range(2):
                        ps = ps_mm.tile([128, FL], f32, tag="psmm")
                        for fi in range(3):
                            nc.tensor.matmul(
                                ps,
                                lhsT=r(yn_sb[:, fi, qt * 128 : (qt + 1) * 128]),
                                rhs=r(wp_sb[:, fi, cj * FL : (cj + 1) * FL]),
                                start=(fi == 0),
                                stop=(fi == 2),
                            )
                        nc.vector.tensor_copy(ostg[:, cj * FL : (cj + 1) * FL], ps)
                    nc.sync.dma_start(
                        out=out[qt * 128 : (qt + 1) * 128, :], in_=ostg
                    )

    nc.compile()
    return nc


def _shard_inputs(x, w_attn, b_attn, w_proj, b_proj):
    in_maps = []
    for core in range(8):
        b, hg = core // 2, core % 2
        hs = hg * FL
        k_w = w_attn[hs : hs + FL]
        q_w = w_attn[C + hs : C + hs + FL]
        v_w = w_attn[2 * C + hs : 2 * C + hs + FL]
        in_maps.append(
            {
                "xT": np.ascontiguousarray(x[b].T),
                "wkqT": np.ascontiguousarray(
                    np.concatenate([k_w, q_w], axis=0).T
                ),
                "wvT": np.ascontiguousarray(v_w.T),
                "bkq": np.ascontiguousarray(
                    np.concatenate([b_attn[hs : hs + FL], b_attn[C + hs : C + hs + FL]])
                ),
                "bv": np.ascontiguousarray(b_attn[2 * C + hs : 2 * C + hs + FL]),
                "wpT": np.ascontiguousarray(w_proj[:, hs : hs + FL].T),
            }
        )
    return in_maps


def _run(inputs, trace=False, trace_kwargs=None):
    from concourse.bass_utils import run_bass_kernel_spmd

    if "nc" not in _cache:
        _cache["nc"] = _build()
    nc = _cache["nc"]
    in_maps = _shard_inputs(**inputs)
    kw = {}
    if trace:
        kw["trace"] = True
        if trace_kwargs:
            kw.update(trace_kwargs)
    res = run_bass_kernel_spmd(nc, in_maps, core_ids=list(range(8)), **kw)
    x = inputs["x"]
    outf = np.empty((B, T, C), dtype=np.float32)
    for b in range(B):
        outf[b] = (
            res.results[2 * b]["out"]
            + res.results[2 * b + 1]["out"]
            + inputs["b_proj"]
        )
    return outf, res


def kernel(x, w_attn, b_attn, w_proj, b_proj):
    assert x.shape == (B, T, C), x.shape
    outf, _ = _run(
        dict(x=x, w_attn=w_attn, b_attn=b_attn, w_proj=w_proj, b_proj=b_proj)
    )
    return outf


# revision 17
# speedup vs baseline: 1.7006x; 1.7006x over previous
"""Causal self-attention (B=4, T=2048, C=768, H=12, D=64) on 8 TRN2 NeuronCores.

Sharding: core = 2*b + hg. Data parallel over batch (4), tensor parallel over
heads (2 groups of 6). Each core computes qkv for its 6 heads, causal
attention, and a partial output projection (its heads' columns of w_proj);
the host sums the two partials per batch and adds b_proj.

Layout notes (per core):
  - xT   [768, 2048]  x[b] transposed on host (contraction dim on partitions)
  - kq   [128, 6, 2048] SBUF: f-tiles 0-2 = K^T feats, 3-5 = Q^T feats
  - v    [128, 16, 390] SBUF: token-major V, 65 cols/head (col 64 = ones so
         the attn@V matmul also produces the softmax denominator l)
  - scores computed transposed S^T[k, q] so no transposes are needed anywhere;
    softmax uses no max-subtraction (logits are O(10) for this problem) so
    P = exp(0.25 * QK^T_raw), Y^T_aug = V_aug^T @ P^T accumulated over k-tiles.
  - matmul operands are bf16 (fp32 PSUM accumulation); the 1/l row uses a
    single custom-DVE reciprocal and a tiny f32r ones-outer-product broadcast.
  - S^T tiles are paired [128, 1024] so one ACT exp covers two k-tiles.
"""
import sys

for _p in ("/opt/trn_rl_repo",):
    if _p not in sys.path:
        sys.path.append(_p)

import numpy as np

B, T, C = 4, 2048, 768
H, D = 12, 64
HL = H // 2          # 6 local heads
FL = HL * D          # 384 local features
NCT = C // 128       # 6 contraction tiles
NTT = T // 128       # 16 token tiles
QCH = 512            # q chunk (free dim of attention matmuls)
NQC = T // QCH       # 4 q chunks
VW = D + 1           # 65: V columns per head incl. ones column
EXP_SCALE = 2.0 / np.sqrt(D)  # reference uses logits = 2 * scores / sqrt(D)

_cache = {}


def _build():
    import concourse.bass as bass
    import concourse.tile as tile
    from concourse import bacc, mybir

    f32 = mybir.dt.float32
    f32r = mybir.dt.float32r
    bf16 = mybir.dt.bfloat16
    Exp = mybir.ActivationFunctionType.Exp

    nc = bacc.Bacc("TRN2", target_bir_lowering=False, debug=False, num_devices=8)

    xT = nc.dram_tensor("xT", [C, T], bf16, kind="ExternalInput").ap()
    wkqT = nc.dram_tensor("wkqT", [C, 2 * FL], bf16, kind="ExternalInput").ap()
    wvT = nc.dram_tensor("wvT", [C, FL], bf16, kind="ExternalInput").ap()
    bkq = nc.dram_tensor("bkq", [2 * FL], f32, kind="ExternalInput").ap()
    bv = nc.dram_tensor("bv", [FL], f32, kind="ExternalInput").ap()
    wpT = nc.dram_tensor("wpT", [FL, C], bf16, kind="ExternalInput").ap()
    out = nc.dram_tensor("out", [T, C], f32, kind="ExternalOutput").ap()

    with tile.TileContext(nc) as tc:
        from contextlib import ExitStack

        with ExitStack() as ctx:
            persist = ctx.enter_context(tc.tile_pool(name="persist", bufs=1))
            xpool = ctx.enter_context(tc.tile_pool(name="xchunk", bufs=2))
            ppool = ctx.enter_context(tc.tile_pool(name="ptile", bufs=3))
            lpool = ctx.enter_context(tc.tile_pool(name="linv", bufs=2))
            lrpool = ctx.enter_context(tc.tile_pool(name="linvrep", bufs=2))
            opool = ctx.enter_context(tc.tile_pool(name="outstg", bufs=2))
            # PSUM: pss 2x2 banks + psy 3x1 + psmm 1x1 = 8 banks
            ps_mm = ctx.enter_context(tc.tile_pool(name="psmm", bufs=1, space="PSUM"))
            ps_s = ctx.enter_context(tc.tile_pool(name="pss", bufs=2, space="PSUM"))
            ps_y = ctx.enter_context(tc.tile_pool(name="psy", bufs=3, space="PSUM"))

            # ---- persistent SBUF tensors ----
            kq_sb = persist.tile([128, 6, T], bf16)         # K^T (0-2) / Q^T (3-5)
            v_sb = persist.tile([128, NTT, HL * VW], bf16)  # token-major V + ones
            yn_sb = persist.tile([128, 3, T], bf16)         # normalized Y^T
            wkq_sb = persist.tile([128, NCT, 2 * FL], bf16)
            wv_sb = persist.tile([128, NCT, FL], bf16)
            wp_sb = persist.tile([128, 3, C], bf16)
            bkq_sb = persist.tile([128, NCT], f32)
            bv_rep = persist.tile([128, FL], f32)
            masks = persist.tile([128, 4, QCH], bf16)
            ones32 = persist.tile([1, 64], f32)
            nc.vector.memset(ones32, 1.0)
            ones_sb = persist.tile([1, 64], f32r)
            nc.vector.tensor_copy(ones_sb, ones32)

            # ---- load weights / biases ----
            nc.sync.dma_start(
                out=wkq_sb, in_=wkqT.rearrange("(a p) f -> p a f", p=128)
            )
            nc.sync.dma_start(out=wv_sb, in_=wvT.rearrange("(a p) f -> p a f", p=128))
            nc.sync.dma_start(out=wp_sb, in_=wpT.rearrange("(a p) f -> p a f", p=128))
            nc.sync.dma_start(out=bkq_sb, in_=bkq.rearrange("(a p) -> p a", p=128))
            nc.gpsimd.dma_start(
                out=bv_rep,
                in_=bass.AP(tensor=bv.tensor, offset=0, ap=[[0, 128], [1, FL]]),
            )

            # ---- causal masks for the 4 diagonal offsets ----
            # tile (kt, qc) is diagonal when mi = kt - qc*4 in {0,1,2,3}; its
            # global offset is qc*512 - kt*128 = -mi*128, and an element
            # (p=k rel, f=q rel) is valid (keep) iff -mi*128 + f - p >= 0.
            masks32 = persist.tile([128, 4, QCH], f32)
            for mi in range(4):
                m = masks32[:, mi, :]
                nc.gpsimd.memset(m, 1.0)
                nc.gpsimd.affine_select(
                    out=m,
                    in_=m,
                    compare_op=mybir.AluOpType.is_ge,
                    fill=0.0,
                    base=-mi * 128,
                    channel_multiplier=-1,
                    pattern=[[1, QCH]],
                )
            nc.vector.tensor_copy(masks, masks32)

            # ones columns of v_sb
            v4 = v_sb.rearrange("p t (h w) -> p t h w", h=HL)
            nc.vector.memset(v4[:, :, :, D : D + 1], 1.0)

            # ---- phase A: qkv ----
            for tn in range(NQC):  # token chunks of 512
                xt = xpool.tile([128, NCT, QCH], bf16)
                nc.sync.dma_start(
                    out=xt,
                    in_=xT.rearrange("(a p) t -> p a t", p=128)[
                        :, :, tn * QCH : (tn + 1) * QCH
                    ],
                )
                # K^T / Q^T: out[f_tile, tok]
                for fj in range(6):
                    ps = ps_mm.tile([128, QCH], f32, tag="psmm")
                    for ci in range(NCT):
                        nc.tensor.matmul(
                            ps,
                            lhsT=wkq_sb[:, ci, fj * 128 : (fj + 1) * 128],
                            rhs=xt[:, ci, :],
                            start=(ci == 0),
                            stop=(ci == NCT - 1),
                        )
                    nc.vector.tensor_scalar_add(
                        kq_sb[:, fj, tn * QCH : (tn + 1) * QCH],
                        ps,
                        bkq_sb[:, fj : fj + 1],
                    )
                # V: out[token, f] (+ bias along free dim)
                for k4 in range(4):
                    kt = tn * 4 + k4
                    ps = ps_mm.tile([128, FL], f32, tag="psmm")
                    for ci in range(NCT):
                        nc.tensor.matmul(
                            ps,
                            lhsT=xt[:, ci, k4 * 128 : (k4 + 1) * 128],
                            rhs=wv_sb[:, ci, :],
                            start=(ci == 0),
                            stop=(ci == NCT - 1),
                        )
                    nc.vector.tensor_add(
                        v4[:, kt, :, 0:D],
                        ps.rearrange("p (h d) -> p h d", h=HL),
                        bv_rep.rearrange("p (h d) -> p h d", h=HL),
                    )

            # ---- phase B: attention, phase C: projection per q-chunk ----
            for qc in range(NQC):
                for h in range(HL):
                    hp, hi = h % 2, h // 2
                    kmax = (qc + 1) * 4
                    psy = ps_y.tile([128, QCH], f32, tag="psy")
                    for kp in range(kmax // 2):
                        pss = ps_s.tile([128, 2 * QCH], f32, tag="pss")
                        for ki in range(2):
                            kt = 2 * kp + ki
                            nc.tensor.matmul(
                                pss[:, ki * QCH : (ki + 1) * QCH],
                                lhsT=kq_sb[
                                    hp * 64 : hp * 64 + 64,
                                    hi,
                                    kt * 128 : (kt + 1) * 128,
                                ],
                                rhs=kq_sb[
                                    hp * 64 : hp * 64 + 64,
                                    3 + hi,
                                    qc * QCH : (qc + 1) * QCH,
                                ],
                                start=True,
                                stop=True,
                            )
                        pt = ppool.tile([128, 2 * QCH], bf16, tag="ptile")
                        nc.scalar.activation(pt, pss, Exp, scale=float(EXP_SCALE))
                        for ki in range(2):
                            kt = 2 * kp + ki
                            mi = kt - qc * 4
                            if mi >= 0:  # diagonal tile -> apply causal mask
                                nc.vector.tensor_mul(
                                    pt[:, ki * QCH : (ki + 1) * QCH],
                                    pt[:, ki * QCH : (ki + 1) * QCH],
                                    masks[:, mi, :],
                                )
                            nc.tensor.matmul(
                                psy[0:VW, :],
                                lhsT=v_sb[:, kt, h * VW : (h + 1) * VW],
                                rhs=pt[:, ki * QCH : (ki + 1) * QCH],
                                start=(kt == 0),
                                stop=(kt == kmax - 1),
                            )
                    lrow = lpool.tile([1, QCH], f32, tag="lrow")
                    nc.vector.tensor_copy(lrow, psy[D : D + 1, :])
                    linv32 = lpool.tile([1, QCH], f32, tag="linv32")
                    nc.vector.reciprocal_approx_fast(out=linv32, in_=lrow)
                    linv = lpool.tile([1, QCH], f32r, tag="linv")
                    nc.vector.tensor_copy(linv, linv32)
                    psb = ps_y.tile([128, QCH], f32, tag="psy")
                    nc.tensor.matmul(
                        psb[0:64, :], lhsT=ones_sb, rhs=linv, start=True, stop=True
                    )
                    lrep = lrpool.tile([64, QCH], f32, tag="linvrep")
                    nc.vector.tensor_copy(lrep, psb[0:64, :])
                    nc.vector.tensor_mul(
                        yn_sb[hp * 64 : hp * 64 + 64, hi, qc * QCH : (qc + 1) * QCH],
                        psy[0:D, :],
                        lrep,
                    )
                # projection for this q-chunk (all heads now normalized)
                for q4 in range(4):
                    qt = qc * 4 + q4
                    ostg = opool.tile([128, C], f32, tag="outstg")
                    for cj in range(2):
                        ps = ps_mm.tile([128, FL], f32, tag="psmm")
                        for fi in range(3):
                            nc.tensor.matmul(
                                ps,
                                lhsT=yn_sb[:, fi, qt * 128 : (qt + 1) * 128],
                                rhs=wp_sb[:, fi, cj * FL : (cj + 1) * FL],
                                start=(fi == 0),
                                stop=(fi == 2),
                            )
                        nc.vector.tensor_copy(ostg[:, cj * FL : (cj + 1) * FL], ps)
                    nc.sync.dma_start(
                        out=out[qt * 128 : (qt + 1) * 128, :], in_=ostg
                    )

    nc.compile()
    return nc


def _shard_inputs(x, w_attn, b_attn, w_proj, b_proj):
    import ml_dtypes

    bf16 = ml_dtypes.bfloat16
    in_maps = []
    for core in range(8):
        b, hg = core // 2, core % 2
        hs = hg * FL
        k_w = w_attn[hs : hs + FL]
        q_w = w_attn[C + hs : C + hs + FL]
        v_w = w_attn[2 * C + hs : 2 * C + hs + FL]
        in_maps.append(
            {
                "xT": np.ascontiguousarray(x[b].T).astype(bf16),
                "wkqT": np.ascontiguousarray(
                    np.concatenate([k_w, q_w], axis=0).T
                ).astype(bf16),
                "wvT": np.ascontiguousarray(v_w.T).astype(bf16),
                "bkq": np.ascontiguousarray(
                    np.concatenate([b_attn[hs : hs + FL], b_attn[C + hs : C + hs + FL]])
                ).astype(np.float32),
                "bv": np.ascontiguousarray(
                    b_attn[2 * C + hs : 2 * C + hs + FL]
                ).astype(np.float32),
                "wpT": np.ascontiguousarray(w_proj[:, hs : hs + FL].T).astype(bf16),
            }
        )
    return in_maps


def _run(inputs, trace=False, trace_kwargs=None):
    from concourse.bass_utils import run_bass_kernel_spmd

    if "nc" not in _cache:
        _cache["nc"] = _build()
    nc = _cache["nc"]
    in_maps = _shard_inputs(**inputs)
    kw = {}
    if trace:
        kw["trace"] = True
        if trace_kwargs:
            kw.update(trace_kwargs)
    res = run_bass_kernel_spmd(nc, in_maps, core_ids=list(range(8)), **kw)
    x = inputs["x"]
    outf = np.empty((B, T, C), dtype=np.float32)
    for b in range(B):
        outf[b] = (
            res.results[2 * b]["out"]
            + res.results[2 * b + 1]["out"]
            + inputs["b_proj"]
        )
    return outf, res


def kernel(x, w_attn, b_attn, w_proj, b_proj):
    assert x.shape == (B, T, C), x.shape
    outf, _ = _run(
        dict(x=x, w_attn=w_attn, b_attn=b_attn, w_proj=w_proj, b_proj=b_proj)
    )
    return outf


# revision 19
# speedup vs baseline: 1.7660x; 1.0385x over previous
"""Causal self-attention (B=4, T=2048, C=768, H=12, D=64) on 8 TRN2 NeuronCores.

Sharding: core = 2*b + hg. Data parallel over batch (4), tensor parallel over
heads (2 groups of 6). Each core computes qkv for its 6 heads, causal
attention, and a partial output projection (its heads' columns of w_proj);
the host sums the two partials per batch and adds b_proj.

Layout notes (per core):
  - xT   [768, 2048]  x[b] transposed on host (contraction dim on partitions)
  - kq   [128, 6, 2048] SBUF: f-tiles 0-2 = K^T feats, 3-5 = Q^T feats
  - v    [128, 16, 390] SBUF: token-major V, 65 cols/head (col 64 = ones so
         the attn@V matmul also produces the softmax denominator l)
  - scores computed transposed S^T[k, q] so no transposes are needed anywhere;
    softmax uses no max-subtraction (logits are O(10) for this problem) so
    P = exp(0.25 * QK^T_raw), Y^T_aug = V_aug^T @ P^T accumulated over k-tiles.
  - matmul operands are bf16 (fp32 PSUM accumulation); 1/l via a custom-DVE
    approx reciprocal (SBUF-in only!) + tiny f32r ones-outer-product broadcast.
  - S^T tiles are paired [128, 1024] so one ACT exp covers two k-tiles.
  - QKV for token-chunk n+1 and projection for chunk n-1 are interleaved into
    attention of chunk n to keep the PE stream dense (softmax is ACT-paced).
"""
import sys

for _p in ("/opt/trn_rl_repo",):
    if _p not in sys.path:
        sys.path.append(_p)

import numpy as np

B, T, C = 4, 2048, 768
H, D = 12, 64
HL = H // 2          # 6 local heads
FL = HL * D          # 384 local features
NCT = C // 128       # 6 contraction tiles
NTT = T // 128       # 16 token tiles
QCH = 512            # q chunk (free dim of attention matmuls)
NQC = T // QCH       # 4 q chunks
VW = D + 1           # 65: V columns per head incl. ones column
EXP_SCALE = 2.0 / np.sqrt(D)  # reference uses logits = 2 * scores / sqrt(D)

_cache = {}


def _build():
    import concourse.bass as bass
    import concourse.tile as tile
    from concourse import bacc, mybir

    f32 = mybir.dt.float32
    f32r = mybir.dt.float32r
    bf16 = mybir.dt.bfloat16
    Exp = mybir.ActivationFunctionType.Exp

    nc = bacc.Bacc("TRN2", target_bir_lowering=False, debug=False, num_devices=8)

    xT = nc.dram_tensor("xT", [C, T], bf16, kind="ExternalInput").ap()
    wkqT = nc.dram_tensor("wkqT", [C, 2 * FL], bf16, kind="ExternalInput").ap()
    wvT = nc.dram_tensor("wvT", [C, FL], bf16, kind="ExternalInput").ap()
    bkq = nc.dram_tensor("bkq", [2 * FL], f32, kind="ExternalInput").ap()
    bv = nc.dram_tensor("bv", [FL], f32, kind="ExternalInput").ap()
    wpT = nc.dram_tensor("wpT", [FL, C], bf16, kind="ExternalInput").ap()
    out = nc.dram_tensor("out", [T, C], f32, kind="ExternalOutput").ap()

    with tile.TileContext(nc) as tc:
        from contextlib import ExitStack

        with ExitStack() as ctx:
            persist = ctx.enter_context(tc.tile_pool(name="persist", bufs=1))
            xpool = ctx.enter_context(tc.tile_pool(name="xchunk", bufs=2))
            ppool = ctx.enter_context(tc.tile_pool(name="ptile", bufs=3))
            lpool = ctx.enter_context(tc.tile_pool(name="linv", bufs=2))
            lrpool = ctx.enter_context(tc.tile_pool(name="linvrep", bufs=2))
            opool = ctx.enter_context(tc.tile_pool(name="outstg", bufs=2))
            # PSUM: psmm 2x1 banks + pss 2x2 + psy 2x1 = 8 banks
            ps_mm = ctx.enter_context(tc.tile_pool(name="psmm", bufs=2, space="PSUM"))
            ps_s = ctx.enter_context(tc.tile_pool(name="pss", bufs=2, space="PSUM"))
            ps_y = ctx.enter_context(tc.tile_pool(name="psy", bufs=2, space="PSUM"))

            # ---- persistent SBUF tensors ----
            kq_sb = persist.tile([128, 6, T], bf16)         # K^T (0-2) / Q^T (3-5)
            v_sb = persist.tile([128, NTT, HL * VW], bf16)  # token-major V + ones
            yn_sb = persist.tile([128, 3, T], bf16)         # normalized Y^T
            wkq_sb = persist.tile([128, NCT, 2 * FL], bf16)
            wv_sb = persist.tile([128, NCT, FL], bf16)
            wp_sb = persist.tile([128, 3, C], bf16)
            bkq_sb = persist.tile([128, NCT], f32)
            bv_rep = persist.tile([128, FL], f32)
            masks = persist.tile([128, 4, QCH], bf16)
            ones32 = persist.tile([1, 64], f32)
            nc.vector.memset(ones32, 1.0)
            ones_sb = persist.tile([1, 64], f32r)
            nc.vector.tensor_copy(ones_sb, ones32)

            # ---- load weights / biases (split per c-tile for fast start) ----
            wkq_r = wkqT.rearrange("(a p) f -> p a f", p=128)
            wv_r = wvT.rearrange("(a p) f -> p a f", p=128)
            for ci in range(NCT):
                nc.sync.dma_start(out=wkq_sb[:, ci, :], in_=wkq_r[:, ci, :])
                nc.sync.dma_start(out=wv_sb[:, ci, :], in_=wv_r[:, ci, :])
            wp_r = wpT.rearrange("(a p) f -> p a f", p=128)
            for fi in range(3):
                nc.sync.dma_start(out=wp_sb[:, fi, :], in_=wp_r[:, fi, :])
            nc.sync.dma_start(out=bkq_sb, in_=bkq.rearrange("(a p) -> p a", p=128))
            nc.gpsimd.dma_start(
                out=bv_rep,
                in_=bass.AP(tensor=bv.tensor, offset=0, ap=[[0, 128], [1, FL]]),
            )

            # ---- causal masks for the 4 diagonal offsets ----
            # tile (kt, qc) is diagonal when mi = kt - qc*4 in {0,1,2,3}; its
            # global offset is qc*512 - kt*128 = -mi*128, and an element
            # (p=k rel, f=q rel) is valid (keep) iff -mi*128 + f - p >= 0.
            # affine_select's predicate iota needs >8 mantissa bits -> build in
            # f32, then convert to bf16 (values are exactly 0/1).
            masks32 = persist.tile([128, 4, QCH], f32)
            for mi in range(4):
                m = masks32[:, mi, :]
                nc.gpsimd.memset(m, 1.0)
                nc.gpsimd.affine_select(
                    out=m,
                    in_=m,
                    compare_op=mybir.AluOpType.is_ge,
                    fill=0.0,
                    base=-mi * 128,
                    channel_multiplier=-1,
                    pattern=[[1, QCH]],
                )
            nc.vector.tensor_copy(masks, masks32)

            # ones columns of v_sb
            v4 = v_sb.rearrange("p t (h w) -> p t h w", h=HL)
            nc.vector.memset(v4[:, :, :, D : D + 1], 1.0)

            xT_r = xT.rearrange("(a p) t -> p a t", p=128)
            x_tiles = {}

            def load_x(tn):
                xt = xpool.tile([128, NCT, QCH], bf16, tag="xchunk", name=f"xt{tn}")
                for ci in range(NCT):
                    nc.sync.dma_start(
                        out=xt[:, ci, :],
                        in_=xT_r[:, ci, tn * QCH : (tn + 1) * QCH],
                    )
                x_tiles[tn] = xt

            def qkv_chains(tn):
                """10 closures: 6 K/Q feature-tile chains + 4 V token-tile chains."""
                chains = []

                def kq_chain(fj, tn=tn):
                    xt = x_tiles[tn]
                    ps = ps_mm.tile([128, QCH], f32, tag="psmm", name=f"kq{tn}_{fj}")
                    for ci in range(NCT):
                        nc.tensor.matmul(
                            ps,
                            lhsT=wkq_sb[:, ci, fj * 128 : (fj + 1) * 128],
                            rhs=xt[:, ci, :],
                            start=(ci == 0),
                            stop=(ci == NCT - 1),
                        )
                    nc.vector.tensor_scalar_add(
                        kq_sb[:, fj, tn * QCH : (tn + 1) * QCH],
                        ps,
                        bkq_sb[:, fj : fj + 1],
                    )

                def v_chain(k4, tn=tn):
                    xt = x_tiles[tn]
                    kt = tn * 4 + k4
                    ps = ps_mm.tile([128, FL], f32, tag="psmm", name=f"v{kt}")
                    for ci in range(NCT):
                        nc.tensor.matmul(
                            ps,
                            lhsT=xt[:, ci, k4 * 128 : (k4 + 1) * 128],
                            rhs=wv_sb[:, ci, :],
                            start=(ci == 0),
                            stop=(ci == NCT - 1),
                        )
                    nc.vector.tensor_add(
                        v4[:, kt, :, 0:D],
                        ps.rearrange("p (h d) -> p h d", h=HL),
                        bv_rep.rearrange("p (h d) -> p h d", h=HL),
                    )

                for fj in range(6):
                    chains.append(lambda fj=fj: kq_chain(fj))
                for k4 in range(4):
                    chains.append(lambda k4=k4: v_chain(k4))
                return chains

            def proj_chains(qc):
                """4 closures, one per token tile of chunk qc."""
                chains = []

                def proj_tile(qt):
                    ostg = opool.tile([128, C], f32, tag="outstg", name=f"o{qt}")
                    for cj in range(2):
                        ps = ps_mm.tile(
                            [128, FL], f32, tag="psmm", name=f"pj{qt}_{cj}"
                        )
                        for fi in range(3):
                            nc.tensor.matmul(
                                ps,
                                lhsT=yn_sb[:, fi, qt * 128 : (qt + 1) * 128],
                                rhs=wp_sb[:, fi, cj * FL : (cj + 1) * FL],
                                start=(fi == 0),
                                stop=(fi == 2),
                            )
                        nc.vector.tensor_copy(ostg[:, cj * FL : (cj + 1) * FL], ps)
                    nc.sync.dma_start(
                        out=out[qt * 128 : (qt + 1) * 128, :], in_=ostg
                    )

                for q4 in range(4):
                    chains.append(lambda qt=qc * 4 + q4: proj_tile(qt))
                return chains

            def attn_unit(qc, h):
                hp, hi = h % 2, h // 2
                kmax = (qc + 1) * 4
                psy = ps_y.tile([128, QCH], f32, tag="psy", name=f"y{qc}_{h}")
                for kp in range(kmax // 2):
                    pss = ps_s.tile(
                        [128, 2 * QCH], f32, tag="pss", name=f"s{qc}_{h}_{kp}"
                    )
                    for ki in range(2):
                        kt = 2 * kp + ki
                        nc.tensor.matmul(
                            pss[:, ki * QCH : (ki + 1) * QCH],
                            lhsT=kq_sb[
                                hp * 64 : hp * 64 + 64,
                                hi,
                                kt * 128 : (kt + 1) * 128,
                            ],
                            rhs=kq_sb[
                                hp * 64 : hp * 64 + 64,
                                3 + hi,
                                qc * QCH : (qc + 1) * QCH,
                            ],
                            start=True,
                            stop=True,
                        )
                    pt = ppool.tile(
                        [128, 2 * QCH], bf16, tag="ptile", name=f"p{qc}_{h}_{kp}"
                    )
                    nc.scalar.activation(pt, pss, Exp, scale=float(EXP_SCALE))
                    for ki in range(2):
                        kt = 2 * kp + ki
                        mi = kt - qc * 4
                        if mi >= 0:  # diagonal tile -> apply causal mask
                            nc.vector.tensor_mul(
                                pt[:, ki * QCH : (ki + 1) * QCH],
                                pt[:, ki * QCH : (ki + 1) * QCH],
                                masks[:, mi, :],
                            )
                        nc.tensor.matmul(
                            psy[0:VW, :],
                            lhsT=v_sb[:, kt, h * VW : (h + 1) * VW],
                            rhs=pt[:, ki * QCH : (ki + 1) * QCH],
                            start=(kt == 0),
                            stop=(kt == kmax - 1),
                        )
                # softmax denominator: lrow -> 1/l -> f32r -> broadcast to 64 rows
                lrow = lpool.tile([1, QCH], f32, tag="lrow", name=f"lr{qc}_{h}")
                nc.vector.tensor_copy(lrow, psy[D : D + 1, :])
                linv32 = lpool.tile([1, QCH], f32, tag="linv32", name=f"li{qc}_{h}")
                nc.vector.reciprocal_approx_fast(out=linv32, in_=lrow)
                linv = lpool.tile([1, QCH], f32r, tag="linv", name=f"lv{qc}_{h}")
                nc.vector.tensor_copy(linv, linv32)
                psb = ps_mm.tile([128, QCH], f32, tag="psmm", name=f"lb{qc}_{h}")
                nc.tensor.matmul(
                    psb[0:64, :], lhsT=ones_sb, rhs=linv, start=True, stop=True
                )
                lrep = lrpool.tile([64, QCH], f32, tag="linvrep", name=f"lp{qc}_{h}")
                nc.vector.tensor_copy(lrep, psb[0:64, :])
                nc.vector.tensor_mul(
                    yn_sb[hp * 64 : hp * 64 + 64, hi, qc * QCH : (qc + 1) * QCH],
                    psy[0:D, :],
                    lrep,
                )

            # ---- pipelined emission ----
            # qkv(qc+1) chains MUST all be emitted within round qc (before any
            # attention unit of round qc+1 reads them), so they get their own
            # queue that is force-drained at round end; proj chains have no
            # such deadline and fill remaining slots.
            load_x(0)
            for chain in qkv_chains(0):
                chain()
            q_fill = []
            p_fill = []
            for qc in range(NQC):
                if qc + 1 < NQC:
                    load_x(qc + 1)
                    q_fill.extend(qkv_chains(qc + 1))
                for h in range(HL):
                    attn_unit(qc, h)
                    for _ in range(2):
                        if q_fill:
                            q_fill.pop(0)()
                        elif p_fill:
                            p_fill.pop(0)()
                while q_fill:
                    q_fill.pop(0)()
                p_fill.extend(proj_chains(qc))
            while p_fill:
                p_fill.pop(0)()

    nc.compile()
    return nc


def _shard_inputs(x, w_attn, b_attn, w_proj, b_proj):
    import ml_dtypes

    bf16 = ml_dtypes.bfloat16
    in_maps = []
    for core in range(8):
        b, hg = core // 2, core % 2
        hs = hg * FL
        k_w = w_attn[hs : hs + FL]
        q_w = w_attn[C + hs : C + hs + FL]
        v_w = w_attn[2 * C + hs : 2 * C + hs + FL]
        in_maps.append(
            {
                "xT": np.ascontiguousarray(x[b].T).astype(bf16),
                "wkqT": np.ascontiguousarray(
                    np.concatenate([k_w, q_w], axis=0).T
                ).astype(bf16),
                "wvT": np.ascontiguousarray(v_w.T).astype(bf16),
                "bkq": np.ascontiguousarray(
                    np.concatenate([b_attn[hs : hs + FL], b_attn[C + hs : C + hs + FL]])
                ).astype(np.float32),
                "bv": np.ascontiguousarray(
                    b_attn[2 * C + hs : 2 * C + hs + FL]
                ).astype(np.float32),
                "wpT": np.ascontiguousarray(w_proj[:, hs : hs + FL].T).astype(bf16),
            }
        )
    return in_maps


def _run(inputs, trace=False, trace_kwargs=None):
    from concourse.bass_utils import run_bass_kernel_spmd

    if "nc" not in _cache:
        _cache["nc"] = _build()
    nc = _cache["nc"]
    in_maps = _shard_inputs(**inputs)
    kw = {}
    if trace:
        kw["trace"] = True
        if trace_kwargs:
            kw.update(trace_kwargs)
    res = run_bass_kernel_spmd(nc, in_maps, core_ids=list(range(8)), **kw)
    x = inputs["x"]
    outf = np.empty((B, T, C), dtype=np.float32)
    for b in range(B):
        outf[b] = (
            res.results[2 * b]["out"]
            + res.results[2 * b + 1]["out"]
            + inputs["b_proj"]
        )
    return outf, res


def kernel(x, w_attn, b_attn, w_proj, b_proj):
    assert x.shape == (B, T, C), x.shape
    outf, _ = _run(
        dict(x=x, w_attn=w_attn, b_attn=b_attn, w_proj=w_proj, b_proj=b_proj)
    )
    return outf
